# revision 1
# baseline (speedup 1.0000x reference)
"""Causal self-attention (B=8, T=1024, C=768, H=12) on 8 TRN2 NeuronCores.

Data-parallel over batch: each core computes one batch element end-to-end
(qkv projection, causal attention, output projection). No collectives.

Self-contained: builds and compiles the Bass program on first call and
caches it for subsequent calls.
"""

import numpy as np

import concourse.bass as bass
import concourse.mybir as mybir
from concourse import bacc
from concourse.tile import TileContext
from concourse.bass_utils import run_bass_kernel_spmd
from concourse.masks import make_identity, make_upper_triangular

f32 = mybir.dt.float32
f32r = mybir.dt.float32r
EXP = mybir.ActivationFunctionType.Exp
COPY = mybir.ActivationFunctionType.Copy

N_CORES = 8
T = 1024          # sequence length (per core batch element)
C = 768           # embedding dim
H = 12            # heads
DK = 64           # head dim
NCC = C // 128    # 6 C-chunks
NTT = T // 128    # 8 token tiles
SCALE = 1.0 / np.sqrt(DK)


def _r(ap):
    return ap if ap.dtype == f32r else ap.bitcast(f32r)


def build_program(qkv_bias: bool, out_bias: bool, iters: int = 1):
    nc = bacc.Bacc("TRN2", num_devices=N_CORES, debug=False)

    x = nc.dram_tensor("x", [T, C], f32, kind="ExternalInput").ap()
    wqkv = nc.dram_tensor("W_qkv", [C, 3 * C], f32r, kind="ExternalInput").ap()
    bqkv = nc.dram_tensor("b_qkv", [3 * C], f32r, kind="ExternalInput").ap()
    wout = nc.dram_tensor("W_out", [C, C], f32r, kind="ExternalInput").ap()
    bout = nc.dram_tensor("b_out", [C], f32r, kind="ExternalInput").ap()
    y = nc.dram_tensor("y", [T, C], f32, kind="ExternalOutput").ap()

    with TileContext(nc) as tc:
        with tc.tile_pool(name="const", bufs=1) as cpool, \
             tc.tile_pool(name="qk", bufs=1) as qkpool, \
             tc.tile_pool(name="vpp", bufs=1) as vppool, \
             tc.tile_pool(name="attn", bufs=1) as atpool:

            ident = cpool.tile([128, 128], f32, tag="ident")
            make_identity(nc, ident)

            # Triangle mask (1 where k<=q within a diagonal 128x128 block)
            tri = cpool.tile([128, 128], f32, tag="tri")
            make_upper_triangular(nc, tri, val=1.0, diag=True)

            if qkv_bias or out_bias:
                ones_row = cpool.tile([1, 512], f32r, tag="ones_row")
                nc.gpsimd.memset(ones_row, 1.0)
            if qkv_bias:
                bq_sb = cpool.tile([1, 3 * C], f32r, tag="bq")
                nc.sync.dma_start(out=bq_sb, in_=bqkv[None, :])
            if out_bias:
                bo_sb = cpool.tile([1, C], f32r, tag="bo")
                nc.sync.dma_start(out=bo_sb, in_=bout[None, :])

            for it in range(iters):
                # Persistent activations (slots shared across bench iterations)
                qkT = [qkpool.tile([128, T], f32r, tag=f"qkT{m}", name=f"qkT{m}") for m in range(12)]
                vp = [vppool.tile([128, H * 65], f32r, tag=f"vp{t}", name=f"vp{t}") for t in range(NTT)]

                # ---------------- Phase A+B: xT, qkT, V' ----------------
                with tc.tile_pool(name=f"xload{it}", bufs=6) as xpool, \
                     tc.tile_pool(name=f"wq{it}", bufs=1) as wqpool, \
                     tc.tile_pool(name=f"xT{it}", bufs=1) as xTpool, \
                     tc.tile_pool(name=f"psA{it}", bufs=3, space="PSUM") as psA, \
                     tc.tile_pool(name=f"psB{it}", bufs=5, space="PSUM") as psB:

                    xT = [xTpool.tile([128, T], f32r, tag=f"xT{c}", name=f"xT{c}") for c in range(NCC)]
                    wq = [wqpool.tile([128, 3 * C], f32r, tag=f"wq{c}", name=f"wq{c}")
                          for c in range(NCC)]
                    for t in range(NTT):
                        xt = xpool.tile([128, C], f32, tag="x")
                        dma = nc.gpsimd if t % 2 else nc.sync
                        dma.dma_start(out=xt, in_=x[t * 128:(t + 1) * 128, :])
                        for c in range(NCC):
                            tp = psA.tile([128, 128], f32, tag="tp")
                            nc.tensor.transpose(tp, xt[:, c * 128:(c + 1) * 128], ident)
                            nc.scalar.activation(
                                xT[c][:, t * 128:(t + 1) * 128], tp, COPY)
                    for c in range(NCC):
                        # split each chunk across both DMA queues to halve latency
                        nc.gpsimd.dma_start(out=wq[c][:, 0:1152],
                                            in_=wqkv[c * 128:(c + 1) * 128, 0:1152])
                        nc.sync.dma_start(out=wq[c][:, 1152:],
                                          in_=wqkv[c * 128:(c + 1) * 128, 1152:])

                    # q, k feature-major: qkT[m] rows = features m*128..m*128+127
                    # (m 0..5 -> q features 0..767, m 6..11 -> k features 0..767)
                    for m in (0, 6, 1, 7, 2, 8, 3, 9, 4, 10, 5, 11):
                        for nj in range(2):
                            ps = psB.tile([128, 512], f32, tag="mm")
                            if qkv_bias:
                                nc.tensor.matmul(
                                    ps, _r(bq_sb[0:1, m * 128:(m + 1) * 128]),
                                    _r(ones_row), start=True, stop=False)
                            for c in range(NCC):
                                nc.tensor.matmul(
                                    ps,
                                    _r(wq[c][:, m * 128:(m + 1) * 128]),
                                    _r(xT[c][:, nj * 512:(nj + 1) * 512]),
                                    start=(c == 0 and not qkv_bias), stop=(c == NCC - 1))
                            nc.vector.tensor_copy(qkT[m][:, nj * 512:(nj + 1) * 512], ps)

                    # v token-major, written into V' with a ones column per head
                    for t in range(NTT):
                        for n0, nw in ((0, 512), (512, 256)):
                            ps = psB.tile([128, 512], f32, tag="mm")
                            psv = ps[:, 0:nw]
                            if qkv_bias:
                                nc.tensor.matmul(
                                    psv, _r(ones_row[0:1, 0:128]),
                                    _r(bq_sb[0:1, 2 * C + n0:2 * C + n0 + nw]),
                                    start=True, stop=False)
                            for c in range(NCC):
                                nc.tensor.matmul(
                                    psv,
                                    _r(xT[c][:, t * 128:(t + 1) * 128]),
                                    _r(wq[c][:, 2 * C + n0:2 * C + n0 + nw]),
                                    start=(c == 0 and not qkv_bias), stop=(c == NCC - 1))
                            dst = vp[t].rearrange("p (h e) -> p h e", e=65)[
                                :, n0 // 64:(n0 + nw) // 64, 0:64]
                            nc.vector.tensor_copy(
                                dst, psv.rearrange("p (a b) -> p a b", b=64))
                        nc.gpsimd.memset(
                            vp[t].rearrange("p (h e) -> p h e", e=65)[:, :, 64:65]
                            .bitcast(f32), 1.0)

                # ---------------- Phase C: attention ----------------
                with tc.tile_pool(name=f"pb{it}", bufs=8) as pbpool, \
                     tc.tile_pool(name=f"rr{it}", bufs=6) as rrpool, \
                     tc.tile_pool(name=f"wo{it}", bufs=1) as wopool:
                  woutT = [wopool.tile([128, C], f32r, tag=f"woutT{c}",
                                       name=f"woutT{c}") for c in range(NCC)]
                  for c in range(NCC):
                      # W_out loads overlap attention on the idle gpsimd queue
                      nc.gpsimd.dma_start(out=woutT[c],
                                          in_=wout[c * 128:(c + 1) * 128, :])
                  with tc.tile_pool(name=f"psS{it}", bufs=3, space="PSUM") as psS, \
                       tc.tile_pool(name=f"psPV{it}", bufs=2, space="PSUM") as psPV:

                    attnT = [atpool.tile([128, T], f32r, tag=f"attnT{c}",
                                         name=f"attnT{c}") for c in range(NCC)]
                    for hp in range(H // 2):
                        # head pair (2hp, 2hp+1): partitions 0:64 / 64:128 of the
                        # same qkT tiles -> S matmuls land in different PE row
                        # groups and overlap on the array.
                        qTt = qkT[hp]
                        kTt = qkT[6 + hp]
                        for qj in range(2):
                            nki = 4 * qj + 4
                            pvs = [psPV.tile([128, 512], f32, tag="pv",
                                             name=f"pv{hp}_{qj}_{e}") for e in range(2)]
                            for g in range(nki // 2):
                                sps = [psS.tile([128, 1024], f32, tag="s",
                                                name=f"s{hp}_{qj}_{g}_{e}")
                                       for e in range(2)]
                                jp0 = g - 2 * qj
                                # per-ki placement in the 1024-col group:
                                # (colbase, o) with o = causally-dead prefix
                                # width that is never computed. The near-band
                                # group stores [j1|j0] so its single dead
                                # prefix sits at the group start and exp can
                                # run as one suffix op.
                                if jp0 == 0:
                                    placement = [(512, 0), (0, 128)]
                                elif jp0 == 1:
                                    # j3's true prefix is 384, but N=128 runs
                                    # at 1/4 fp32r rate (= N=256 cost); widen
                                    # to N=256 - the extra columns are never
                                    # read (exp skips them, pb is zeroed)
                                    placement = [(0, 256), (512, 256)]
                                else:
                                    placement = [(0, 0), (512, 0)]
                                for loc in range(2):
                                    ki = g * 2 + loc
                                    cb, o = placement[loc]
                                    for e in range(2):
                                        qb = e * 64
                                        nc.tensor.matmul(
                                            sps[e][:, cb + o:cb + 512],
                                            _r(kTt[qb:qb + 64,
                                                   ki * 128:(ki + 1) * 128]),
                                            _r(qTt[qb:qb + 64,
                                                   qj * 512 + o:(qj + 1) * 512]),
                                            start=True, stop=True)
                                jp = jp0
                                pbs = []
                                for e in range(2):
                                    pb = pbpool.tile([128, 1024], f32r, tag="pb",
                                                     name=f"pb{hp}_{qj}_{g}_{e}")
                                    if jp == 1:
                                        # one strided op over both valid-ish
                                        # 256-col blocks ([256:512) and
                                        # [768:1024)); the dead [768:896) part
                                        # is finite and memset-zeroed after
                                        nc.scalar.activation(
                                            pb.rearrange("p (a b) -> p a b",
                                                         b=256)[:, 1:4:2, :],
                                            sps[e].rearrange("p (a b) -> p a b",
                                                             b=256)[:, 1:4:2, :],
                                            EXP, scale=float(SCALE))
                                    elif jp == 0:
                                        nc.scalar.activation(
                                            pb[:, 128:1024], sps[e][:, 128:1024],
                                            EXP, scale=float(SCALE))
                                    else:
                                        nc.scalar.activation(pb, sps[e], EXP,
                                                             scale=float(SCALE))
                                    pbs.append(pb)
                                if jp == 0:
                                    # layout [j1|j0]: zero the j1 prefix,
                                    # triangles at [128:256) (j1) and
                                    # [512:640) (j0)
                                    for e in range(2):
                                        pb = pbs[e]
                                        nc.gpsimd.memset(
                                            pb[:, 0:128].bitcast(f32), 0.0)
                                        nc.vector.tensor_mul(
                                            pb[:, 128:256], pb[:, 128:256], tri)
                                        nc.vector.tensor_mul(
                                            pb[:, 512:640], pb[:, 512:640], tri)
                                elif jp == 1:
                                    for e in range(2):
                                        pb = pbs[e]
                                        nc.gpsimd.memset(
                                            pb[:, 0:256].bitcast(f32), 0.0)
                                        nc.vector.tensor_mul(
                                            pb[:, 256:384], pb[:, 256:384], tri)
                                        nc.gpsimd.memset(
                                            pb[:, 512:896].bitcast(f32), 0.0)
                                        nc.vector.tensor_mul(
                                            pb[:, 896:1024], pb[:, 896:1024], tri)
                                # PV: stream only valid columns, clamped to
                                # N>=256 (below that fp32r runs at 1/4 rate so
                                # narrower costs the same). Emit in placement
                                # order with ki ascending per column region so
                                # the start=True (ki==0) matmul executes first.
                                for loc in range(2):
                                    ki = g * 2 + loc
                                    cb, o = placement[loc]
                                    ov = min(o, 256)
                                    for e in range(2):
                                        h = 2 * hp + e
                                        nc.tensor.matmul(
                                            pvs[e][0:65, ov:512],
                                            _r(vp[ki][:, h * 65:(h + 1) * 65]),
                                            _r(pbs[e][:, cb + ov:cb + 512]),
                                            start=(ki == 0), stop=(ki == nki - 1))
                            for e in range(2):
                                qb = e * 64
                                # copy PV out of PSUM promptly to release the
                                # bank for the next group's matmuls
                                pvsb = rrpool.tile([65, 512], f32, tag="pvsb",
                                                   name=f"pvsb{hp}_{qj}_{e}")
                                nc.vector.tensor_copy(pvsb, pvs[e][0:65, :])
                                recip = rrpool.tile([1, 512], f32, tag="recip",
                                                    name=f"recip{hp}_{qj}_{e}")
                                nc.vector.reciprocal(recip, pvsb[64:65, :])
                                rb = rrpool.tile([64, 512], f32, tag="rb",
                                                 name=f"rb{hp}_{qj}_{e}")
                                nc.gpsimd.partition_broadcast(rb, recip)
                                nc.vector.tensor_mul(
                                    attnT[hp][qb:qb + 64, qj * 512:(qj + 1) * 512],
                                    pvsb[0:64, :], rb)

                  # -------------- Phase D: output projection --------------
                  with tc.tile_pool(name=f"yst{it}", bufs=4) as ypool, \
                       tc.tile_pool(name=f"psO{it}", bufs=4, space="PSUM") as psO:
                    for t in range(NTT):
                        for n0, nw in ((0, 512), (512, 256)):
                            ps = psO.tile([128, 512], f32, tag="o")
                            pso = ps[:, 0:nw]
                            if out_bias:
                                nc.tensor.matmul(
                                    pso, _r(ones_row[0:1, 0:128]),
                                    _r(bo_sb[0:1, n0:n0 + nw]), start=True, stop=False)
                            for c in range(NCC):
                                nc.tensor.matmul(
                                    pso,
                                    _r(attnT[c][:, t * 128:(t + 1) * 128]),
                                    _r(woutT[c][:, n0:n0 + nw]),
                                    start=(c == 0 and not out_bias), stop=(c == NCC - 1))
                            ysb = ypool.tile([128, 512], f32, tag="y")
                            nc.vector.tensor_copy(ysb[:, 0:nw], pso)
                            nc.sync.dma_start(
                                out=y[t * 128:(t + 1) * 128, n0:n0 + nw],
                                in_=ysb[:, 0:nw])

    nc.compile()
    return nc


_CACHE = {}


def _get_program(qkv_bias: bool, out_bias: bool):
    key = (qkv_bias, out_bias)
    if key not in _CACHE:
        _CACHE[key] = build_program(qkv_bias, out_bias)
    return _CACHE[key]


def _make_in_maps(x, W_qkv, b_qkv, W_out, b_out):
    x = np.ascontiguousarray(np.asarray(x, dtype=np.float32))
    W_qkv = np.ascontiguousarray(np.asarray(W_qkv, dtype=np.float32))
    b_qkv = np.ascontiguousarray(np.asarray(b_qkv, dtype=np.float32))
    W_out = np.ascontiguousarray(np.asarray(W_out, dtype=np.float32))
    b_out = np.ascontiguousarray(np.asarray(b_out, dtype=np.float32))
    return [
        {"x": x[i], "W_qkv": W_qkv, "b_qkv": b_qkv, "W_out": W_out, "b_out": b_out}
        for i in range(N_CORES)
    ]


def kernel(x, W_qkv, b_qkv, W_out, b_out):
    qkv_bias = bool(np.any(np.asarray(b_qkv)))
    out_bias = bool(np.any(np.asarray(b_out)))
    nc = _get_program(qkv_bias, out_bias)
    in_maps = _make_in_maps(x, W_qkv, b_qkv, W_out, b_out)
    res = run_bass_kernel_spmd(nc, in_maps, core_ids=list(range(N_CORES)))
    return np.stack([res.results[i]["y"] for i in range(N_CORES)], axis=0)


def bench(x, W_qkv, b_qkv, W_out, b_out, trace=True):
    """Run with NTFF tracing; returns (output, BassKernelResults)."""
    qkv_bias = bool(np.any(np.asarray(b_qkv)))
    out_bias = bool(np.any(np.asarray(b_out)))
    nc = _get_program(qkv_bias, out_bias)
    in_maps = _make_in_maps(x, W_qkv, b_qkv, W_out, b_out)
    res = run_bass_kernel_spmd(nc, in_maps, core_ids=list(range(N_CORES)),
                               trace=trace)
    out = np.stack([res.results[i]["y"] for i in range(N_CORES)], axis=0)
    return out, res



# revision 27
# speedup vs baseline: 1.1293x; 1.1293x over previous
"""Causal self-attention (B=8, T=1024, C=768, H=12) on 8 TRN2 NeuronCores.

Data-parallel over batch: each core computes one batch element end-to-end.

Design notes (cost-model driven):
- Matmul cost = moving-operand rows; 16-bit movers run 1 cycle/row at any
  width, f32r needs N>=256. Weights stay f32r (stationary side is free).
- PV is computed "flipped" (q-tokens on output partitions, N=65 per head):
  28080 rows instead of 55296, and the softmax denominator becomes a
  per-partition scalar, normalized on the idle GPSIMD engine
  (normalize_recip) instead of DVE recip/broadcast/mul.
- All transposes (x -> xT, attn -> attnT) go through the DMA crossbar
  (dma_start_transpose, 14ns/16x128 tile) instead of the PE array.
- S/exp/PV widths are causally exact; the diagonal 128-block is masked by a
  two-block strided DVE multiply after exp.
- Emission is software-pipelined: QKV / V / out-proj matmul chains are used
  as PE filler between the S -> exp -> PV rounds so the tensor engine never
  head-of-line blocks on the activation engine.
"""

import numpy as np

import concourse.bass as bass
import concourse.mybir as mybir
from concourse import bacc
from concourse.tile import TileContext
from concourse.bass_utils import run_bass_kernel_spmd
from concourse.masks import make_identity, make_upper_triangular

f32 = mybir.dt.float32
f32r = mybir.dt.float32r
bf16 = mybir.dt.bfloat16
fp16 = mybir.dt.float16
EXP = mybir.ActivationFunctionType.Exp
COPY = mybir.ActivationFunctionType.Copy

N_CORES = 8
T = 1024
C = 768
H = 12
DK = 64
NCC = C // 128      # 6 feature chunks
NTT = T // 128      # 8 token tiles
HP = H // 2         # 6 head pairs
SCALE = 1.0 / np.sqrt(DK)


def _r(ap):
    return ap if ap.dtype == f32r else ap.bitcast(f32r)


def build_program(qkv_bias: bool, out_bias: bool, dbg: bool = False):
    nc = bacc.Bacc("TRN2", num_devices=N_CORES, debug=False)

    x = nc.dram_tensor("x", [T, C], f32r, kind="ExternalInput").ap()
    wqkv = nc.dram_tensor("W_qkv", [C, 3 * C], f32r, kind="ExternalInput").ap()
    bqkv = nc.dram_tensor("b_qkv", [3 * C], f32r, kind="ExternalInput").ap()
    wout = nc.dram_tensor("W_out", [C, C], f32r, kind="ExternalInput").ap()
    bout = nc.dram_tensor("b_out", [C], f32r, kind="ExternalInput").ap()
    y = nc.dram_tensor("y", [T, C], f32, kind="ExternalOutput").ap()
    if dbg:
        t_xT = nc.dram_tensor("t_xT", [128, NCC * T], f32, kind="ExternalOutput").ap()
        t_qkT0 = nc.dram_tensor("t_qkT0", [128, T], fp16, kind="ExternalOutput").ap()
        t_qkT6 = nc.dram_tensor("t_qkT6", [128, T], fp16, kind="ExternalOutput").ap()
        t_vp0 = nc.dram_tensor("t_vp0", [128, H * 65], bf16, kind="ExternalOutput").ap()
        t_asb0 = nc.dram_tensor("t_asb0", [128, C], fp16, kind="ExternalOutput").ap()
        t_asb7 = nc.dram_tensor("t_asb7", [128, C], fp16, kind="ExternalOutput").ap()
        t_attnT = nc.dram_tensor("t_attnT", [128, NCC * T], fp16, kind="ExternalOutput").ap()

    with TileContext(nc) as tc:
        with tc.tile_pool(name="const", bufs=1) as cpool, \
             tc.tile_pool(name="wq", bufs=1) as wqpool, \
             tc.tile_pool(name="wo", bufs=1) as wopool, \
             tc.tile_pool(name="xT", bufs=1) as xTpool, \
             tc.tile_pool(name="qk", bufs=1) as qkpool, \
             tc.tile_pool(name="vp", bufs=1) as vppool, \
             tc.tile_pool(name="atn", bufs=1) as atpool, \
             tc.tile_pool(name="asb", bufs=4) as asbpool, \
             tc.tile_pool(name="xst", bufs=3) as xstpool, \
             tc.tile_pool(name="pb", bufs=12) as pbpool, \
             tc.tile_pool(name="pvs", bufs=3) as pvspool, \
             tc.tile_pool(name="ysb", bufs=2) as ypool, \
             tc.tile_pool(name="psS", bufs=2, space="PSUM") as psS, \
             tc.tile_pool(name="psPV", bufs=1, space="PSUM") as psPV, \
             tc.tile_pool(name="psB", bufs=2, space="PSUM") as psB:

            # triangle mask, duplicated so one strided DVE op covers both
            # heads' diagonal blocks: tri2[:, 0:128] == tri2[:, 128:256]
            trif = cpool.tile([128, 128], f32, tag="trif")
            make_upper_triangular(nc, trif, val=1.0, diag=True)
            tri2 = cpool.tile([128, 256], bf16, tag="tri2")
            nc.vector.tensor_copy(tri2[:, 0:128], trif)
            nc.vector.tensor_copy(tri2[:, 128:256], trif)
            tri2v = tri2.rearrange("p (a b) -> p a b", b=128)
            identf = cpool.tile([128, 128], f32, tag="identf")
            make_identity(nc, identf)
            identr = cpool.tile([128, 128], f32r, tag="identr")
            nc.scalar.activation(identr, identf, COPY)
            identh = cpool.tile([128, 128], fp16, tag="identh")
            nc.vector.tensor_copy(identh, identf)

            if qkv_bias or out_bias:
                ones_row = cpool.tile([1, 512], f32r, tag="ones_row")
                nc.gpsimd.memset(ones_row, 1.0)
            if qkv_bias:
                bq_sb = cpool.tile([1, 3 * C], f32r, tag="bq")
                nc.sync.dma_start(out=bq_sb, in_=bqkv[None, :])
            if out_bias:
                bo_sb = cpool.tile([1, C], f32r, tag="bo")
                nc.sync.dma_start(out=bo_sb, in_=bout[None, :])

            # ---------------- persistent tiles ----------------
            wq = [wqpool.tile([128, 3 * C], f32r, tag=f"wq{c}", name=f"wq{c}")
                  for c in range(NCC)]
            woutT = [wopool.tile([128, C], f32r, tag=f"woT{c}", name=f"woT{c}")
                     for c in range(NCC)]
            woutT16 = [wopool.tile([128, C], fp16, tag=f"woT16_{c}",
                                   name=f"woT16_{c}") for c in range(NCC)]
            xT = xTpool.tile([128, NCC * T], f32r, tag="xT", name="xT")
            xTv = xT.rearrange("p (c t) -> p c t", t=T)
            qkT = [qkpool.tile([128, T], fp16, tag=f"qkT{m}", name=f"qkT{m}")
                   for m in range(12)]
            vp = [vppool.tile([128, H * 65], bf16, tag=f"vp{t}", name=f"vp{t}")
                  for t in range(NTT)]
            attnT = atpool.tile([128, NCC * T], fp16, tag="attnT", name="attnT")
            attnTv = attnT.rearrange("p (c t) -> p c t", t=T)
            attn_sb = [asbpool.tile([128, C], fp16, tag="asb", name=f"asb{j}")
                       for j in range(NTT)]

            # ---------------- input DMA (arrival-ordered) ----------------
            # Single SP queue preserves issue order; global DMA engines
            # serialize transfers, so order = arrival priority:
            # x0-3 (transpose path), q|k weight cols, x4-7, v weight cols.
            xf = [xstpool.tile([128, C], f32r, tag="xf", name=f"xf{t}")
                  for t in range(NTT)]
            for t in range(4):
                nc.sync.dma_start(out=xf[t], in_=x[t * 128:(t + 1) * 128, :])
            for lo_col, hi_col in ((0, 384), (768, 1152)):
                for c in range(NCC):
                    nc.sync.dma_start(out=wq[c][:, lo_col:hi_col],
                                      in_=wqkv[c * 128:(c + 1) * 128,
                                               lo_col:hi_col])
            for t in range(4, 8):
                nc.sync.dma_start(out=xf[t], in_=x[t * 128:(t + 1) * 128, :])
            for lo_col, hi_col in ((384, 768), (1152, 1536)):
                for c in range(NCC):
                    nc.sync.dma_start(out=wq[c][:, lo_col:hi_col],
                                      in_=wqkv[c * 128:(c + 1) * 128,
                                               lo_col:hi_col])
            for c in range(NCC):
                nc.sync.dma_start(out=wq[c][:, 1536:],
                                  in_=wqkv[c * 128:(c + 1) * 128, 1536:])
            for c in range(NCC):
                nc.sync.dma_start(out=woutT[c],
                                  in_=wout[c * 128:(c + 1) * 128, :])

            # x -> xT via PE transposes (bf16 identity mover = 1 cyc/row),
            # strided Pool copies convert f32 PSUM -> fp16 xT
            def x_transpose(t):
                tp1 = psB.tile([128, 512], f32, tag="g", name=f"xp{t}a")
                tp2 = psB.tile([128, 512], f32, tag="g", name=f"xp{t}b")
                for c in range(NCC):
                    dst = (tp1 if c < 4 else tp2)[:, (c % 4) * 128:
                                                  (c % 4) * 128 + 128]
                    nc.tensor.matmul(
                        dst.bitcast(f32r),
                        xf[t][:, c * 128:(c + 1) * 128],
                        identr, is_transpose=True,
                        start=(c % 4 == 0), stop=(c % 4 == 3 or c == 5),
                        skip_group_check=True)
                nc.scalar.activation(
                    xTv[:, 0:4, t * 128:(t + 1) * 128],
                    tp1.rearrange("p (a b) -> p a b", b=128), COPY)
                nc.scalar.activation(
                    xTv[:, 4:6, t * 128:(t + 1) * 128],
                    tp2.rearrange("p (a b) -> p a b", b=128)[:, 0:2, :], COPY)

            # ---------------- filler chain generators ----------------
            def qk_chain(m, nj):
                """q/k projection for feature chunk m (0-5 q, 6-11 k)."""
                ps = psB.tile([128, 512], f32, tag="g", name=f"qk{m}_{nj}")
                if qkv_bias:
                    nc.tensor.matmul(ps, _r(bq_sb[0:1, m * 128:(m + 1) * 128]),
                                     _r(ones_row), start=True, stop=False)
                for c in range(NCC):
                    nc.tensor.matmul(
                        ps, wq[c][:, m * 128:(m + 1) * 128],
                        xTv[:, c, nj * 512:(nj + 1) * 512],
                        start=(c == 0 and not qkv_bias), stop=(c == NCC - 1))
                    yield
                nc.vector.tensor_copy(qkT[m][:, nj * 512:(nj + 1) * 512], ps)
                yield

            def v_chain(t):
                """v projection for token tile t, into vp (65-per-head + ones)."""
                for n0, nw in ((0, 512), (512, 256)):
                    ps = psB.tile([128, 512], f32, tag="g", name=f"v{t}_{n0}")
                    psv = ps[:, 0:nw]
                    if qkv_bias:
                        nc.tensor.matmul(
                            psv, _r(ones_row[0:1, 0:128]),
                            _r(bq_sb[0:1, 2 * C + n0:2 * C + n0 + nw]),
                            start=True, stop=False)
                    for c in range(NCC):
                        nc.tensor.matmul(
                            psv, xTv[:, c, t * 128:(t + 1) * 128],
                            wq[c][:, 2 * C + n0:2 * C + n0 + nw],
                            start=(c == 0 and not qkv_bias), stop=(c == NCC - 1))
                        yield
                    dst = vp[t].rearrange("p (h e) -> p h e", e=65)[
                        :, n0 // 64:(n0 + nw) // 64, 0:64]
                    nc.vector.tensor_copy(
                        dst, psv.rearrange("p (a b) -> p a b", b=64))
                    yield
                nc.gpsimd.memset(
                    vp[t].rearrange("p (h e) -> p h e", e=65)[:, :, 64:65], 1.0)
                yield

            def oproj_chain(t):
                """output projection + store for token tile t."""
                ysb = ypool.tile([128, C], f32, tag="y", name=f"ysb{t}")
                for n0, nw in ((0, 512), (512, 256)):
                    ps = psB.tile([128, 512], f32, tag="g", name=f"o{t}_{n0}")
                    pso = ps[:, 0:nw]
                    if out_bias:
                        nc.tensor.matmul(
                            pso, _r(ones_row[0:1, 0:128]),
                            _r(bo_sb[0:1, n0:n0 + nw]), start=True, stop=False)
                    for c in range(NCC):
                        nc.tensor.matmul(
                            pso, attnTv[:, c, t * 128:(t + 1) * 128],
                            woutT16[c][:, n0:n0 + nw],
                            start=(c == 0 and not out_bias), stop=(c == NCC - 1))
                        yield
                    nc.vector.tensor_copy(ysb[:, n0:n0 + nw], pso)
                    yield
                    nc.sync.dma_start(out=y[t * 128:(t + 1) * 128, n0:n0 + nw],
                                      in_=ysb[:, n0:n0 + nw])
                yield

            fillers = []

            def pump(n):
                done = 0
                while fillers and done < n:
                    try:
                        next(fillers[0])
                        done += 1
                    except StopIteration:
                        fillers.pop(0)

            def drain(gen):
                for _ in gen:
                    pass

            # ---------------- attention block ----------------
            def s_phase(hp, qj, budget=0):
                """S matmuls + exp + diag mask for all ki; returns pb dict."""
                nki = 4 * (qj + 1)
                pbs = {}
                for ki in range(nki):
                    emit_s(hp, qj, ki, pbs)
                    pump(budget)
                return pbs

            def emit_s(hp, qj, ki, pbs):
                ps = psS.tile([128, 1024], f32, tag="s", name=f"s{hp}_{qj}_{ki}")
                lo = max(0, ki * 128 - qj * 512)
                for e in range(2):
                    nc.tensor.matmul(
                        ps[:, e * 512 + lo:(e + 1) * 512],
                        qkT[6 + hp][e * 64:(e + 1) * 64, ki * 128:(ki + 1) * 128],
                        qkT[hp][e * 64:(e + 1) * 64,
                                qj * 512 + lo:(qj + 1) * 512],
                        start=True, stop=True)
                pb_t = pbpool.tile([128, 1024], bf16, tag="pb",
                                   name=f"pb{hp}_{qj}_{ki}")
                nc.scalar.activation(
                    pb_t.rearrange("p (a b) -> p a b", b=512)[:, :, lo:512],
                    ps.rearrange("p (a b) -> p a b", b=512)[:, :, lo:512],
                    EXP, scale=float(SCALE))
                if ki >= 4 * qj:  # this k-tile contains the diagonal
                    nc.vector.tensor_mul(
                        pb_t.rearrange("p (a b) -> p a b", b=512)
                        [:, :, lo:lo + 128],
                        pb_t.rearrange("p (a b) -> p a b", b=512)
                        [:, :, lo:lo + 128],
                        tri2v)
                pbs[ki] = pb_t

            def attn_transpose(j):
                for c in range(NCC):
                    tp = psB.tile([128, 512], f32, tag="g", name=f"at{j}_{c}")
                    tph = tp[:, 0:64].bitcast(fp16)
                    nc.tensor.matmul(
                        tph, attn_sb[j][:, c * 128:(c + 1) * 128],
                        identh, is_transpose=True)
                    nc.vector.tensor_copy(
                        attnTv[:, c, j * 128:(j + 1) * 128], tph)

            def emit_pv(hp, qj, ki, pbs, pv):
                pb_t = pbs.pop(ki)
                for jj in range(max(0, ki - 4 * qj), 4):
                    j = 4 * qj + jj
                    for e in range(2):
                        h = 2 * hp + e
                        nc.tensor.matmul(
                            pv[:, jj * 256 + e * 65:jj * 256 + e * 65 + 65],
                            pb_t[:, e * 512 + jj * 128:e * 512 + (jj + 1) * 128],
                            vp[ki][:, h * 65:(h + 1) * 65],
                            start=(ki == 0 and e == 0 and jj in (0, 2)),
                            stop=(ki == j), skip_group_check=True)

            def norm_jj(hp, qj, pv, jj, after_norm=None):
                j = 4 * qj + jj
                pvsb = pvspool.tile([128, 130], f32, tag="pvsb",
                                    name=f"pvsb{hp}_{qj}_{jj}")
                nc.vector.tensor_copy(pvsb, pv[:, jj * 256:jj * 256 + 130])
                for e in range(2):
                    h = 2 * hp + e
                    nc.gpsimd.normalize_recip(
                        attn_sb[j][:, h * 64:(h + 1) * 64],
                        pvsb[:, e * 65:e * 65 + 64],
                        pvsb[:, e * 65 + 64:e * 65 + 65])
                if after_norm is not None:
                    after_norm(j)

            def pv_phase(hp, qj, pbs, budget=0, after_norm=None):
                nki = 4 * (qj + 1)
                pv = psPV.tile([128, 1024], f32, tag="pv", name=f"pv{hp}_{qj}")
                for ki in range(nki):
                    emit_pv(hp, qj, ki, pbs, pv)
                    if ki >= 4 * qj:
                        norm_jj(hp, qj, pv, ki - 4 * qj, after_norm)
                    pump(budget)

            def attn_block(hp, qj, budget, after_norm=None):
                """Software-pipelined S/exp/PV for head pair hp, window qj."""
                nki = 4 * (qj + 1)
                pv = psPV.tile([128, 1024], f32, tag="pv", name=f"pva{hp}_{qj}")
                pbs = {}
                for ki in range(nki):
                    emit_s(hp, qj, ki, pbs)
                    pump(budget)
                    if ki >= 1:
                        emit_pv(hp, qj, ki - 1, pbs, pv)
                        if ki - 1 >= 4 * qj:
                            norm_jj(hp, qj, pv, ki - 1 - 4 * qj, after_norm)
                emit_pv(hp, qj, nki - 1, pbs, pv)
                norm_jj(hp, qj, pv, 3, after_norm)

            # ---------------- main schedule ----------------
            for t in range(4):
                x_transpose(t)
            drain(qk_chain(0, 0))
            drain(qk_chain(1, 0))
            drain(qk_chain(2, 0))
            drain(qk_chain(6, 0))

            def xpose_gen(t):
                x_transpose(t)
                yield

            fillers.extend([qk_chain(7, 0), xpose_gen(4), xpose_gen(5)])
            pbs0 = s_phase(0, 0, budget=6)
            pump(10 ** 6)
            fillers.extend([qk_chain(8, 0), xpose_gen(6), xpose_gen(7)])
            pbs1 = s_phase(1, 0, budget=6)
            pump(10 ** 6)
            pbs2 = s_phase(2, 0)
            for m in (3, 9, 4, 10, 5, 11):
                drain(qk_chain(m, 0))
            for t in range(4):
                drain(v_chain(t))

            for c in range(NCC):
                nc.gpsimd.tensor_copy(woutT16[c], woutT[c])
            pv_phase(0, 0, pbs0)
            pv_phase(1, 0, pbs1)
            pv_phase(2, 0, pbs2)
            for t in range(4, 8):
                drain(v_chain(t))

            attn_block(3, 0, budget=1)
            attn_block(4, 0, budget=1)
            attn_block(5, 0, budget=1, after_norm=attn_transpose)

            fillers.extend([oproj_chain(0), oproj_chain(1),
                            oproj_chain(2), oproj_chain(3)])
            qj1_budget = {0: 1, 1: 1, 2: 1, 3: 2, 4: 3}
            for hp in range(6):
                drain(qk_chain(hp, 1))
                drain(qk_chain(6 + hp, 1))
                if hp < 5:
                    attn_block(hp, 1, budget=qj1_budget[hp])

            def last_norm(j):
                attn_transpose(j)
                fillers.append(oproj_chain(j))

            attn_block(5, 1, budget=3, after_norm=last_norm)
            pump(10 ** 6)
            if dbg:
                nc.sync.dma_start(out=t_xT, in_=xT.bitcast(f32))
                nc.sync.dma_start(out=t_qkT0, in_=qkT[0])
                nc.sync.dma_start(out=t_qkT6, in_=qkT[6])
                nc.sync.dma_start(out=t_vp0, in_=vp[0])
                nc.sync.dma_start(out=t_asb0, in_=attn_sb[0])
                nc.sync.dma_start(out=t_asb7, in_=attn_sb[7])
                nc.sync.dma_start(out=t_attnT, in_=attnT)

    nc.compile()
    return nc


_CACHE = {}


def _get_program(qkv_bias: bool, out_bias: bool):
    key = (qkv_bias, out_bias)
    if key not in _CACHE:
        _CACHE[key] = build_program(qkv_bias, out_bias)
    return _CACHE[key]


def _make_in_maps(x, W_qkv, b_qkv, W_out, b_out):
    x = np.ascontiguousarray(np.asarray(x, dtype=np.float32))
    W_qkv = np.ascontiguousarray(np.asarray(W_qkv, dtype=np.float32))
    b_qkv = np.ascontiguousarray(np.asarray(b_qkv, dtype=np.float32))
    W_out = np.ascontiguousarray(np.asarray(W_out, dtype=np.float32))
    b_out = np.ascontiguousarray(np.asarray(b_out, dtype=np.float32))
    return [
        {"x": x[i], "W_qkv": W_qkv, "b_qkv": b_qkv, "W_out": W_out, "b_out": b_out}
        for i in range(N_CORES)
    ]


def kernel(x, W_qkv, b_qkv, W_out, b_out):
    qkv_bias = bool(np.any(np.asarray(b_qkv)))
    out_bias = bool(np.any(np.asarray(b_out)))
    nc = _get_program(qkv_bias, out_bias)
    in_maps = _make_in_maps(x, W_qkv, b_qkv, W_out, b_out)
    res = run_bass_kernel_spmd(nc, in_maps, core_ids=list(range(N_CORES)))
    return np.stack([res.results[i]["y"] for i in range(N_CORES)], axis=0)


def bench(x, W_qkv, b_qkv, W_out, b_out, trace=True):
    """Run with NTFF tracing; returns (output, BassKernelResults)."""
    qkv_bias = bool(np.any(np.asarray(b_qkv)))
    out_bias = bool(np.any(np.asarray(b_out)))
    nc = _get_program(qkv_bias, out_bias)
    in_maps = _make_in_maps(x, W_qkv, b_qkv, W_out, b_out)
    res = run_bass_kernel_spmd(nc, in_maps, core_ids=list(range(N_CORES)),
                               trace=trace)
    out = np.stack([res.results[i]["y"] for i in range(N_CORES)], axis=0)
    return out, res
